# revision 1
# baseline (speedup 1.0000x reference)
"""BiMatchLoss kernel for Trainium2 (8 NeuronCores, SPMD data-parallel over batch).

Math (validated vs reference):
  BCE(p,t) = -log1mp(p) - t*(logp(p) - log1mp(p))
  Summed over a bijective matching perm, the -log1mp part is perm-independent.
  Per batch b the device computes (one pass over the data):
    cost[t,o]  = -sum_{s,ci} tgt[s,t,ci] * out[s,o,ci]            (argmin input)
    G[t,o]     =  sum_{s,ci} (m[s]*tgt[s,t,ci]) * D'[s,o,ci]
    Amask      =  sum_{s,o,ci} m[s] * (-log1mp[s,o,ci])
  where D' = logp - m*log1mp equals the logit wherever the mask is 1; masked
  rows are zeroed by the host-premasked targets (m*tgt). The mask products
  (m*tgt, m*out) are exact host-side preprocessing (bf16-exact binaries).
  final = sum_b 0.5*(Amask_b - sum_t G[t, perm_b[t]]) / sum(m)

Device per batch: 2 fused Ln ops (ACT; the log(1-x) op also yields the Amask
row-sums via accum_out), one fused D' subtract (DVE), 32 bf16 matmuls (K=128
per s-tile, PSUM-accumulated over 8 tiles, one accumulation group per PSUM
bank), block-diag mask + grouped reduce -> [128,24] partials. Batches are
software-pipelined (prep of b+1 issued before matmuls of b). Host does the
720-permutation argmin and final scalar assembly.
"""

import os
from itertools import permutations

import numpy as np
import ml_dtypes

import concourse.bacc as bacc
import concourse.mybir as mybir
from concourse.tile import TileContext
from concourse.bass_utils import run_bass_kernel_spmd

B, S, E, C = 32, 1024, 6, 16
F = E * C * 2          # 192 flattened (e, c, i)
CI = C * 2             # 32
NCORE = 8
NB = B // NCORE        # 4 batches per core
NT = S // 128          # 8 s-tiles per batch

f32 = mybir.dt.float32
bf16 = mybir.dt.bfloat16
fp8 = mybir.dt.float8e4
AF = mybir.ActivationFunctionType
ALU = mybir.AluOpType
AX = mybir.AxisListType

_PROG = None           # cached compiled Bass program
LAST = None            # last BassKernelResults (for test.py timing)


def _build_program():
    nc = bacc.Bacc("TRN2", target_bir_lowering=False, debug=False,
                   num_devices=1)

    xo_d = nc.dram_tensor("xo", [NB, S, F], bf16, kind="ExternalInput").ap()
    xoo_d = nc.dram_tensor("xoo", [NB, S, F], bf16, kind="ExternalInput").ap()
    xoz_d = nc.dram_tensor("xoz", [NB, S, F], bf16, kind="ExternalInput").ap()
    xt_d = nc.dram_tensor("xt", [NB, S, F], fp8, kind="ExternalInput").ap()
    dmask_d = nc.dram_tensor("dmask", [128, 768], bf16,
                             kind="ExternalInput").ap()
    red_d = nc.dram_tensor("red", [NB, 128, 24], f32,
                           kind="ExternalOutput").ap()
    amask_d = nc.dram_tensor("amask", [NB, 128], f32,
                             kind="ExternalOutput").ap()

    with TileContext(nc) as tc:
        with (
            tc.tile_pool(name="consts", bufs=1) as cpool,
            tc.tile_pool(name="io", bufs=3) as iop,
            tc.tile_pool(name="mid", bufs=3) as midp,
            tc.tile_pool(name="post", bufs=2) as postp,
            tc.tile_pool(name="ps", bufs=3, space="PSUM") as psp,
        ):
            dmask_sb = cpool.tile([128, 768], bf16)
            nc.sync.dma_start(dmask_sb[:], dmask_d)

            def load_tiled(tag, src, dt, eng):
                """DRAM [S,F] -> SBUF [128, NT*F], col block k = s-tile k.
                One DMA per tensor (internally split across 16 SDMA slots);
                eng picks the HWDGE queue (sync vs scalar) for parallelism."""
                t = iop.tile([128, NT * F], dt, tag=tag, name=tag)
                tv = t[:].rearrange("p (k f) -> p k f", f=F)
                sv = src.rearrange("(k p) f -> p k f", p=128)
                eng.dma_start(tv[:], sv[:])
                return t

            def prep(b):
                """Loads + logs + masked-logit + Amask accum for batch b.
                comb holds per-tile [out_k | m*D_k] 384-col blocks: the
                shared matmul rhs."""
                comb = iop.tile([128, NT * 384], bf16, tag="comb", name="comb")
                comb_v = comb[:].rearrange("p (k q) -> p k q", q=384)
                xo_b = xo_d[b].rearrange("(k p) f -> p k f", p=128)
                nc.sync.dma_start(comb_v[:, :, 0:F], xo_b[:])
                xoo_f = load_tiled("xoo_f", xoo_d[b], bf16, nc.scalar)
                xoz_f = load_tiled("xoz_f", xoz_d[b], bf16, nc.scalar)
                xt_f = load_tiled("xt_f", xt_d[b], fp8, nc.sync)

                # logs: cols 0:1536 = m*log(p)  (Ln(m*p + 1-m) = 0 at m=0)
                #       cols 1536:3072 = m*log(1-p); accum -> Amask partials
                logs = midp.tile([128, 2 * NT * F], bf16, tag="logs")
                am_col = postp.tile([128, 1], f32, tag="am_col")
                nc.scalar.activation(logs[:, 0:1536], xoo_f[:], AF.Ln)
                nc.scalar.activation(logs[:, 1536:3072], xoz_f[:], AF.Ln,
                                     bias=1.0, scale=-1.0,
                                     accum_out=am_col[:])
                nc.sync.dma_start(amask_d[b, :], am_col[:])
                # m*D = m*logp - m*log1mp -> comb cols k*384+192:+384
                nc.vector.tensor_sub(comb_v[:, :, F:384], logs[:, 0:1536],
                                     logs[:, 1536:3072])
                return comb, xt_f

            def mms(b, comb, xt_f):
                # 2 matmuls per s-tile (shared xt weights, N=384 rhs =
                # [out_k | m*D_k]), accumulated over the 8 tiles; one
                # accumulation group per PSUM bank:
                #   bank0 [128, 0:384]   = [cost-hi | G-hi]   (M=128)
                #   bank1 [0:64, 512:896] = [cost-lo | G-lo]  (M=64)
                ps = psp.tile([128, 1024], f32, tag="ps")
                nc.vector.memset(ps[64:128, 512:896], 0.0)
                for k in range(NT):
                    st = dict(start=(k == 0), stop=(k == NT - 1))
                    rhs = comb[:, k * 384:(k + 1) * 384]
                    nc.tensor.matmul(ps[:, 0:384],
                                     xt_f[:, k * F:k * F + 128], rhs, **st)
                    nc.tensor.matmul(ps[0:64, 512:896],
                                     xt_f[:, k * F + 128:(k + 1) * F], rhs,
                                     **st)
                return ps

            def post(b, ps):
                # block-diag extraction -> [128, 24] partials
                ps_v = ps[:].rearrange("p (h q) -> p h q", q=512)[:, :, 0:384]
                tmp = postp.tile([128, 768], bf16, tag="tmp")
                nc.vector.tensor_tensor(tmp[:], ps_v, dmask_sb[:], ALU.mult)
                red_sb = postp.tile([128, 24], f32, tag="red_sb")
                nc.vector.tensor_reduce(
                    red_sb[:], tmp[:].rearrange("p (g j) -> p g j", j=CI),
                    AX.X, ALU.add)
                nc.sync.dma_start(red_d[b], red_sb[:])

            state = prep(0)
            pss = None
            for b in range(NB):
                nxt = prep(b + 1) if b + 1 < NB else None
                ps = mms(b, *state)
                post(b, ps)
                state = nxt

    nc.compile()
    return nc


def _get_program():
    global _PROG
    if _PROG is None:
        _PROG = _build_program()
    return _PROG


def kernel(outputs, targets, attention_mask):
    global LAST
    out_np = np.asarray(outputs, dtype=np.float32)
    tgt_np = np.asarray(targets, dtype=np.float32)
    m_np = np.asarray(attention_mask)

    mf = m_np.astype(np.float32)[:, :, None]
    xo_all = out_np.reshape(B, S, F).astype(ml_dtypes.bfloat16)
    # masked copies are exact in bf16 (x*1 or 0); binary targets are exact
    # even in fp8e4
    xoo_all = (out_np.reshape(B, S, F) * mf + (1.0 - mf)).astype(
        ml_dtypes.bfloat16)
    xoz_all = (out_np.reshape(B, S, F) * mf).astype(ml_dtypes.bfloat16)
    xt_all = tgt_np.reshape(B, S, F).astype(ml_dtypes.float8_e4m3fn)

    # dmask[p, q] = 1 where p%32 == q%32 (block-diagonal selector)
    p_idx = np.arange(128)[:, None] % CI
    q_idx = np.arange(768)[None, :] % CI
    dmask = (p_idx == q_idx).astype(ml_dtypes.bfloat16)

    in_maps = []
    for c in range(NCORE):
        bs = slice(c * NB, (c + 1) * NB)
        in_maps.append({
            "xo": np.ascontiguousarray(xo_all[bs]),
            "xoo": np.ascontiguousarray(xoo_all[bs]),
            "xoz": np.ascontiguousarray(xoz_all[bs]),
            "xt": np.ascontiguousarray(xt_all[bs]),
            "dmask": dmask,
        })

    nc = _get_program()
    res = run_bass_kernel_spmd(nc, in_maps, list(range(NCORE)))
    LAST = res

    P = np.array(list(permutations(range(E))), dtype=np.int32)
    t_idx = np.arange(E)[None, :]
    ar = np.arange(E)
    num = 0.0
    for c in range(NCORE):
        red = res.results[c]["red"]      # [NB, 128, 24] f32
        am = res.results[c]["amask"]     # [NB, 128] f32
        for b in range(NB):
            rb = red[b]
            # groups 0:6 cost-hi (rows t0..3 x j), 6:12 G-hi,
            #        12:18 cost-lo (rows 0:64 = t4,5 x j), 18:24 G-lo
            cost = -np.concatenate(
                [rb[:, 0:6].reshape(4, 32, 6).sum(1, dtype=np.float32),
                 rb[0:64, 12:18].reshape(2, 32, 6).sum(1, dtype=np.float32)],
                axis=0)
            G = np.concatenate(
                [rb[:, 6:12].reshape(4, 32, 6).sum(1, dtype=np.float32),
                 rb[0:64, 18:24].reshape(2, 32, 6).sum(1, dtype=np.float32)],
                axis=0)

            totals = cost[t_idx, P].sum(-1, dtype=np.float32)
            perm = P[int(np.argmin(totals))]
            amask_b = -am[b].sum(dtype=np.float64)
            num += 0.5 * (amask_b - float(G[ar, perm].sum(dtype=np.float64)))

    den = float(m_np.sum())
    return np.float32(num / den)



# revision 4
# speedup vs baseline: 1.7142x; 1.7142x over previous
"""BiMatchLoss kernel for Trainium2 (8 NeuronCores, SPMD data-parallel over batch).

Math (validated vs reference):
  BCE(p,t) = -log1mp(p) - t*(logp(p) - log1mp(p))
  Summed over a bijective matching perm, the -log1mp part is perm-independent.
  Per batch b the device computes (one pass over the data):
    cost[t,o]  = -sum_{s,ci} tgt[s,t,ci] * out[s,o,ci]            (argmin input)
    G[t,o]     =  sum_{s,ci} tgt[s,t,ci] * mD[s,o,ci]
    Amask      =  sum_{s,o,ci} m[s] * (-log1mp[s,o,ci])
  where mD = m*(logp - log1mp). Host pre-masks the Ln inputs so the device
  computes m*logp = Ln(m*p + 1-m) and m*log1mp = Ln(m*(1-p) + 1-m) directly
  (the (1-p) form keeps fp8 inputs accurate where p -> 1).
  final = sum_b 0.5*(Amask_b - sum_t G[t, perm_b[t]]) / sum(m)

Device per batch: 2 Ln activations (ACT; the log1mp one accumulates the Amask
row-sums), one DVE subtract writing fp8 mD into the comb rhs slots, 8 fp8
DoubleRow matmuls (K=256 = two s-tiles per matmul, PSUM-accumulated over 4
double-tiles), block-diag mask multiply + grouped reduce -> [128,24] partials.
All input DMAs are contiguous per-partition lines issued up front; one output
DMA returns all partials + Amask accums. Host does the 720-permutation argmin
and final scalar assembly.
"""

import os
from itertools import permutations

import numpy as np
import ml_dtypes

import concourse.bacc as bacc
import concourse.mybir as mybir
from concourse.tile import TileContext
from concourse.bass_utils import run_bass_kernel_spmd

B, S, E, C = 32, 1024, 6, 16
F = E * C * 2          # 192 flattened (e, c, i)
CI = C * 2             # 32
NCORE = 8
NB = B // NCORE        # 4 batches per core
NT = S // 128          # 8 s-tiles per batch
ND = NT // 2           # 4 double-tiles (K=256) per batch

f32 = mybir.dt.float32
bf16 = mybir.dt.bfloat16
fp8 = mybir.dt.float8e4
AF = mybir.ActivationFunctionType
ALU = mybir.AluOpType
AX = mybir.AxisListType
DR = mybir.MatmulPerfMode.DoubleRow

_PROG = None           # cached compiled Bass program
LAST = None            # last BassKernelResults (for test.py timing)

RED_STRIDE = 25        # per-batch cols in red_sb: 12 hi + 12 lo + 1 amask


def _build_program():
    nc = bacc.Bacc("TRN2", target_bir_lowering=False, debug=False,
                   num_devices=1)

    xoo_d = nc.dram_tensor("xoo", [128, NB * 1536], fp8,
                           kind="ExternalInput").ap()
    xzo_d = nc.dram_tensor("xzo", [128, NB * 1536], fp8,
                           kind="ExternalInput").ap()
    xt_d = nc.dram_tensor("xt", [128, NB * 1536], fp8,
                          kind="ExternalInput").ap()
    xoc_d = nc.dram_tensor("xoc", [128, NB * 3072], fp8,
                           kind="ExternalInput").ap()
    dmask_d = nc.dram_tensor("dmask", [128, 384], bf16,
                             kind="ExternalInput").ap()
    red_d = nc.dram_tensor("red", [128, NB * RED_STRIDE], f32,
                           kind="ExternalOutput").ap()

    with TileContext(nc) as tc:
        with (
            tc.tile_pool(name="sb", bufs=1) as sbp,
            tc.tile_pool(name="ps", bufs=1, space="PSUM") as psp,
        ):
            dmask_sb = sbp.tile([128, 384], bf16, tag="dmask")
            red_sb = sbp.tile([128, NB * RED_STRIDE], f32, tag="red")

            xoo_sb, xzo_sb, xt_sb, comb_sb, logs_sb, tmp_sb, ps_sb = (
                [], [], [], [], [], [], [])
            for b in range(NB):
                xoo_sb.append(sbp.tile([128, 1536], fp8, tag=f"xoo{b}", name=f"xoo{b}"))
                xzo_sb.append(sbp.tile([128, 1536], fp8, tag=f"xzo{b}", name=f"xzo{b}"))
                xt_sb.append(sbp.tile([128, 1536], fp8, tag=f"xt{b}", name=f"xt{b}"))
                comb_sb.append(sbp.tile([128, 3072], fp8, tag=f"comb{b}", name=f"comb{b}"))
                logs_sb.append(sbp.tile([128, 3072], bf16, tag=f"logs{b}", name=f"logs{b}"))
                tmp_sb.append(sbp.tile([128, 768], bf16, tag=f"tmp{b}", name=f"tmp{b}"))
                ps_sb.append(psp.tile([128, 1024], f32, tag=f"ps{b}", name=f"ps{b}"))

            # ---- phase A: all input DMAs, queued before any compute op so
            # the in-order engine queues never stall a later dispatch.
            for b in range(NB):
                sl = slice(b * 1536, (b + 1) * 1536)
                nc.sync.dma_start(xoo_sb[b][:], xoo_d[:, sl])
                nc.sync.dma_start(xzo_sb[b][:], xzo_d[:, sl])
                nc.gpsimd.dma_start(comb_sb[b][:],
                                    xoc_d[:, b * 3072:(b + 1) * 3072])
                nc.gpsimd.dma_start(xt_sb[b][:], xt_d[:, sl])
                if b == 1:
                    nc.gpsimd.dma_start(dmask_sb[:], dmask_d)

            # ---- phase B: per-batch compute
            def post(b):
                ps = ps_sb[b]
                t_hi = tmp_sb[b][:, 0:384]
                t_lo = tmp_sb[b][0:64, 384:768]
                nc.vector.tensor_tensor(t_hi, ps[:, 0:384], dmask_sb[:],
                                        ALU.mult)
                nc.vector.tensor_tensor(t_lo, ps[0:64, 512:896],
                                        dmask_sb[0:64, :], ALU.mult)
                c0 = b * RED_STRIDE
                nc.vector.tensor_reduce(
                    red_sb[:, c0:c0 + 12],
                    t_hi.rearrange("p (g j) -> p g j", j=CI), AX.X, ALU.add)
                nc.vector.tensor_reduce(
                    red_sb[0:64, c0 + 12:c0 + 24],
                    t_lo.rearrange("p (g j) -> p g j", j=CI), AX.X, ALU.add)

            for b in range(NB):
                logs = logs_sb[b]
                am_col = red_sb[:, b * RED_STRIDE + 24:b * RED_STRIDE + 25]
                # m*logp = Ln(m*p + 1-m);  m*log1mp = Ln(m*(1-p) + 1-m)
                nc.scalar.activation(logs[:, 0:1536], xoo_sb[b][:], AF.Ln)
                nc.scalar.activation(logs[:, 1536:3072], xzo_sb[b][:], AF.Ln,
                                     accum_out=am_col)
                # mD -> fp8 comb slots (cols 192:384 of each 384-col block)
                comb_v = comb_sb[b][:].rearrange("p (k q) -> p k q", q=384)
                nc.vector.tensor_sub(comb_v[:, :, F:384], logs[:, 0:1536],
                                     logs[:, 1536:3072])

                # 8 DoubleRow matmuls: K=256 (two s-tiles), rhs [128,2,384]
                xt_v = xt_sb[b][:].rearrange("p (k f) -> p k f", f=F)
                ps = ps_sb[b]
                for d in range(ND):
                    st = dict(start=(d == 0), stop=(d == ND - 1))
                    rhs = comb_v[:, 2 * d:2 * d + 2, :]
                    nc.tensor.matmul(ps[:, 0:384],
                                     xt_v[:, 2 * d:2 * d + 2, 0:128], rhs,
                                     perf_mode=DR, **st)
                    nc.tensor.matmul(ps[0:64, 512:896],
                                     xt_v[:, 2 * d:2 * d + 2, 128:F], rhs,
                                     perf_mode=DR, **st)
                if b > 0:
                    post(b - 1)
            post(NB - 1)

            # ---- phase C: single output DMA
            nc.sync.dma_start(red_d, red_sb[:])

    nc.compile()
    return nc


def _get_program():
    global _PROG
    if _PROG is None:
        _PROG = _build_program()
    return _PROG


def _tile_major(x):
    """[NB,S,F'] -> [128, NB*NT*F'] with cols ordered (b, k, f)."""
    nb, s, f = x.shape
    return np.ascontiguousarray(
        x.reshape(nb, NT, 128, f).transpose(2, 0, 1, 3).reshape(128, nb * NT * f))


def kernel(outputs, targets, attention_mask):
    global LAST
    out_np = np.asarray(outputs, dtype=np.float32).reshape(B, S, F)
    tgt_np = np.asarray(targets, dtype=np.float32).reshape(B, S, F)
    m_np = np.asarray(attention_mask)

    mf = m_np.astype(np.float32)[:, :, None]
    f8 = ml_dtypes.float8_e4m3fn
    # masked Ln inputs; binaries and masked copies are cheap host prep
    xoo_all = (out_np * mf + (1.0 - mf)).astype(f8)
    xzo_all = ((1.0 - out_np) * mf + (1.0 - mf)).astype(f8)
    xt_all = tgt_np.astype(f8)
    # comb image: xo tiles in cols 0:192 of each 384 block, zeros elsewhere
    xoc_all = np.zeros((B, NT, 128, 384), dtype=f8)
    xoc_all[:, :, :, 0:F] = out_np.astype(f8).reshape(B, NT, 128, F)

    # dmask[p, q] = 1 where p%32 == q%32 (block-diagonal selector)
    p_idx = np.arange(128)[:, None] % CI
    q_idx = np.arange(384)[None, :] % CI
    dmask = (p_idx == q_idx).astype(ml_dtypes.bfloat16)

    in_maps = []
    for c in range(NCORE):
        bs = slice(c * NB, (c + 1) * NB)
        in_maps.append({
            "xoo": _tile_major(xoo_all[bs]),
            "xzo": _tile_major(xzo_all[bs]),
            "xt": _tile_major(xt_all[bs]),
            "xoc": np.ascontiguousarray(
                xoc_all[bs].transpose(2, 0, 1, 3).reshape(128, NB * 3072)),
            "dmask": dmask,
        })

    nc = _get_program()
    res = run_bass_kernel_spmd(nc, in_maps, list(range(NCORE)))
    LAST = res

    P = np.array(list(permutations(range(E))), dtype=np.int32)
    t_idx = np.arange(E)[None, :]
    ar = np.arange(E)
    num = 0.0
    for c in range(NCORE):
        red = res.results[c]["red"]      # [128, NB*25] f32
        for b in range(NB):
            c0 = b * RED_STRIDE
            hi = red[:, c0:c0 + 12]          # rows (t0..3 x ci) x (o | o)
            lo = red[0:64, c0 + 12:c0 + 24]  # rows (t4,5 x ci)
            cost = -np.concatenate(
                [hi[:, 0:6].reshape(4, CI, 6).sum(1, dtype=np.float32),
                 lo[:, 0:6].reshape(2, CI, 6).sum(1, dtype=np.float32)],
                axis=0)
            G = np.concatenate(
                [hi[:, 6:12].reshape(4, CI, 6).sum(1, dtype=np.float32),
                 lo[:, 6:12].reshape(2, CI, 6).sum(1, dtype=np.float32)],
                axis=0)

            totals = cost[t_idx, P].sum(-1, dtype=np.float32)
            perm = P[int(np.argmin(totals))]
            amask_b = -red[:, c0 + 24].sum(dtype=np.float64)
            num += 0.5 * (amask_b - float(G[ar, perm].sum(dtype=np.float64)))

    den = float(m_np.sum())
    return np.float32(num / den)


# revision 5
# speedup vs baseline: 1.7844x; 1.0410x over previous
"""BiMatchLoss kernel for Trainium2 (8 NeuronCores, SPMD data-parallel over batch).

Math (validated vs reference):
  BCE(p,t) = -log1mp(p) - t*(logp(p) - log1mp(p))
  Summed over a bijective matching perm, the -log1mp part is perm-independent.
  Per batch b the device computes (one pass over the data):
    cost[t,o]  = -sum_{s,ci} tgt[s,t,ci] * out[s,o,ci]            (argmin input)
    G[t,o]     =  sum_{s,ci} tgt[s,t,ci] * mD[s,o,ci]
    Amask      =  sum_{s,o,ci} m[s] * (-log1mp[s,o,ci])
  where mD = m*(logp - log1mp). Host pre-masks the Ln inputs so the device
  computes m*logp = Ln(m*p + 1-m) and m*log1mp = Ln(m*(1-p) + 1-m) directly
  (the (1-p) form keeps fp8 inputs accurate where p -> 1).
  final = sum_b 0.5*(Amask_b - sum_t G[t, perm_b[t]]) / sum(m)

Device per batch: 2 Ln activations over [128,1536] fp8 inputs (ACT; the
log1mp one accumulates Amask row-sums), one DVE subtract writing fp8 mD into
the comb rhs slots, 8 fp8 DoubleRow matmuls (K=256 = two s-tiles per matmul,
PSUM-accumulated over 4 double-tiles), then one fused block-diag mask multiply
+ grouped reduce -> [128,24] partials (bank-1 rows 64:128 are zeroed once at
start so hi/lo extract in a single pair of DVE ops). The Ln inputs arrive as
one [xoo_b|xzo_b]-ordered tensor on the sync queue in exact consumption order
so the ACT chain never stalls; comb xo-slots/xt/dmask ride the gpsimd queue.
Batch 3's log1mp + subtract are split in halves to shorten the serial tail.
Host does the 720-permutation argmin and final scalar assembly.
"""

import os
from itertools import permutations

import numpy as np
import ml_dtypes

import concourse.bacc as bacc
import concourse.mybir as mybir
from concourse.tile import TileContext
from concourse.bass_utils import run_bass_kernel_spmd

B, S, E, C = 32, 1024, 6, 16
F = E * C * 2          # 192 flattened (e, c, i)
CI = C * 2             # 32
NCORE = 8
NB = B // NCORE        # 4 batches per core
NT = S // 128          # 8 s-tiles per batch
ND = NT // 2           # 4 double-tiles (K=256) per batch

f32 = mybir.dt.float32
bf16 = mybir.dt.bfloat16
fp8 = mybir.dt.float8e4
AF = mybir.ActivationFunctionType
ALU = mybir.AluOpType
AX = mybir.AxisListType
DR = mybir.MatmulPerfMode.DoubleRow

_PROG = None           # cached compiled Bass program
LAST = None            # last BassKernelResults (for test.py timing)

RS = 26                # per-batch cols in red_sb: 24 partials + 2 amask


def _build_program():
    nc = bacc.Bacc("TRN2", target_bir_lowering=False, debug=False,
                   num_devices=1)

    lnin_d = nc.dram_tensor("lnin", [128, NB * 3072], fp8,
                            kind="ExternalInput").ap()
    xt_d = nc.dram_tensor("xt", [128, NB * 1536], fp8,
                          kind="ExternalInput").ap()
    xoc_d = nc.dram_tensor("xoc", [128, NB * 1536], fp8,
                           kind="ExternalInput").ap()
    dmask_d = nc.dram_tensor("dmask", [128, 768], bf16,
                             kind="ExternalInput").ap()
    red_d = nc.dram_tensor("red", [128, NB * RS], f32,
                           kind="ExternalOutput").ap()

    with TileContext(nc) as tc:
        with (
            tc.tile_pool(name="sb", bufs=1) as sbp,
            tc.tile_pool(name="ps", bufs=1, space="PSUM") as psp,
        ):
            dmask_sb = sbp.tile([128, 768], bf16, tag="dmask")
            red_sb = sbp.tile([128, NB * RS], f32, tag="red")

            lnin_sb, xt_sb, comb_sb, logs_sb, tmp_sb, ps_sb = (
                [], [], [], [], [], [])
            for b in range(NB):
                lnin_sb.append(sbp.tile([128, 3072], fp8, tag=f"lnin{b}",
                                        name=f"lnin{b}"))
                xt_sb.append(sbp.tile([128, 1536], fp8, tag=f"xt{b}",
                                      name=f"xt{b}"))
                comb_sb.append(sbp.tile([128, 3072], fp8, tag=f"comb{b}",
                                        name=f"comb{b}"))
                logs_sb.append(sbp.tile([128, 3072], bf16, tag=f"logs{b}",
                                        name=f"logs{b}"))
                tmp_sb.append(sbp.tile([128, 768], bf16, tag=f"tmp{b}",
                                       name=f"tmp{b}"))
                ps_sb.append(psp.tile([128, 1024], f32, tag=f"ps{b}",
                                      name=f"ps{b}"))

            # ---- phase A: input DMAs, queued before any compute op.
            # sync carries the Ln inputs in exact ACT consumption order;
            # gpsimd (otherwise idle) carries everything else.
            for b in range(NB):
                nc.sync.dma_start(lnin_sb[b][:],
                                  lnin_d[:, b * 3072:(b + 1) * 3072])
            nc.gpsimd.dma_start(dmask_sb[:], dmask_d)
            comb_vs = []
            for b in range(NB):
                comb_v = comb_sb[b][:].rearrange("p (k q) -> p k q", q=384)
                comb_vs.append(comb_v)
                nc.gpsimd.dma_start(xt_sb[b][:],
                                    xt_d[:, b * 1536:(b + 1) * 1536])
                nc.gpsimd.dma_start(
                    comb_v[:, :, 0:F],
                    xoc_d[:, b * 1536:(b + 1) * 1536].rearrange(
                        "p (k f) -> p k f", f=F))

            # zero the never-written PSUM rows (bank1 rows 64:128) once, and
            # the amask cols, so the fused posts / out-DMA read no
            # uninitialized data
            for b in range(NB):
                nc.vector.memset(ps_sb[b][64:128, 512:896], 0.0)
            nc.vector.memset(red_sb[:], 0.0)

            # ---- phase B: per-batch compute
            def mms(b, dlo, dhi):
                xt_v = xt_sb[b][:].rearrange("p (k f) -> p k f", f=F)
                ps = ps_sb[b]
                for d in range(dlo, dhi):
                    st = dict(start=(d == 0), stop=(d == ND - 1))
                    rhs = comb_vs[b][:, 2 * d:2 * d + 2, :]
                    nc.tensor.matmul(ps[:, 0:384],
                                     xt_v[:, 2 * d:2 * d + 2, 0:128], rhs,
                                     perf_mode=DR, **st)
                    nc.tensor.matmul(ps[0:64, 512:896],
                                     xt_v[:, 2 * d:2 * d + 2, 128:F], rhs,
                                     perf_mode=DR, **st)

            def post(b):
                ps_v = ps_sb[b][:].rearrange(
                    "p (h q) -> p h q", q=512)[:, :, 0:384]
                nc.vector.tensor_tensor(tmp_sb[b][:], ps_v, dmask_sb[:],
                                        ALU.mult)
                nc.vector.tensor_reduce(
                    red_sb[:, b * RS:b * RS + 24],
                    tmp_sb[b][:].rearrange("p (g j) -> p g j", j=CI),
                    AX.X, ALU.add)

            def sub(b, lo, hi):
                logs = logs_sb[b]
                nc.vector.tensor_sub(comb_vs[b][:, lo // F:hi // F, F:384],
                                     logs[:, lo:hi],
                                     logs[:, 1536 + lo:1536 + hi])

            for b in range(NB):
                logs = logs_sb[b]
                lnin = lnin_sb[b]
                nc.scalar.activation(logs[:, 0:1536], lnin[:, 0:1536], AF.Ln)
                if b < NB - 1:
                    nc.scalar.activation(
                        logs[:, 1536:3072], lnin[:, 1536:3072], AF.Ln,
                        accum_out=red_sb[:, b * RS + 24:b * RS + 25])
                    sub(b, 0, 1536)
                    mms(b, 0, ND)
                else:
                    # split the last batch's log1mp/sub/matmuls in halves to
                    # shorten the serial tail after the ACT chain ends
                    nc.scalar.activation(
                        logs[:, 1536:2304], lnin[:, 1536:2304], AF.Ln,
                        accum_out=red_sb[:, b * RS + 24:b * RS + 25])
                    sub(b, 0, 768)
                    mms(b, 0, ND // 2)
                    nc.scalar.activation(
                        logs[:, 2304:3072], lnin[:, 2304:3072], AF.Ln,
                        accum_out=red_sb[:, b * RS + 25:b * RS + 26])
                    sub(b, 768, 1536)
                    mms(b, ND // 2, ND)
                if b > 0:
                    post(b - 1)
            post(NB - 1)

            # ---- phase C: single output DMA
            nc.sync.dma_start(red_d, red_sb[:])

    nc.compile()
    return nc


def _get_program():
    global _PROG
    if _PROG is None:
        _PROG = _build_program()
    return _PROG


def _tile_major(x):
    """[NB,S,F'] -> [128, NB*NT*F'] with cols ordered (b, k, f)."""
    nb, s, f = x.shape
    return np.ascontiguousarray(
        x.reshape(nb, NT, 128, f).transpose(2, 0, 1, 3).reshape(
            128, nb * NT * f))


def kernel(outputs, targets, attention_mask):
    global LAST
    out_np = np.asarray(outputs, dtype=np.float32).reshape(B, S, F)
    tgt_np = np.asarray(targets, dtype=np.float32).reshape(B, S, F)
    m_np = np.asarray(attention_mask)

    mf = m_np.astype(np.float32)[:, :, None]
    f8 = ml_dtypes.float8_e4m3fn
    # masked Ln inputs; binaries and masked copies are cheap host prep.
    # lnin = [xoo_b | xzo_b] per batch, in exact ACT consumption order.
    xoo_all = (out_np * mf + (1.0 - mf)).astype(f8)
    xzo_all = ((1.0 - out_np) * mf + (1.0 - mf)).astype(f8)
    lnin_all = np.concatenate(
        [xoo_all.reshape(B, 1, NT, 128, F),
         xzo_all.reshape(B, 1, NT, 128, F)], axis=1)  # [B, 2, NT, 128, F]
    xt_all = tgt_np.astype(f8)
    xoc_all = out_np.astype(f8)

    # dmask[p, q] = 1 where p%32 == q%32 (block-diagonal selector)
    p_idx = np.arange(128)[:, None] % CI
    q_idx = np.arange(768)[None, :] % CI
    dmask = (p_idx == q_idx).astype(ml_dtypes.bfloat16)

    in_maps = []
    for c in range(NCORE):
        bs = slice(c * NB, (c + 1) * NB)
        in_maps.append({
            "lnin": np.ascontiguousarray(
                lnin_all[bs].transpose(3, 0, 1, 2, 4).reshape(128, NB * 3072)),
            "xt": _tile_major(xt_all[bs]),
            "xoc": _tile_major(xoc_all[bs]),
            "dmask": dmask,
        })

    nc = _get_program()
    res = run_bass_kernel_spmd(nc, in_maps, list(range(NCORE)))
    LAST = res

    P = np.array(list(permutations(range(E))), dtype=np.int32)
    t_idx = np.arange(E)[None, :]
    ar = np.arange(E)
    num = 0.0
    for c in range(NCORE):
        red = res.results[c]["red"]      # [128, NB*RS] f32
        for b in range(NB):
            c0 = b * RS
            hi = red[:, c0:c0 + 12]          # rows (t0..3 x ci) x (o | o)
            lo = red[0:64, c0 + 12:c0 + 24]  # rows (t4,5 x ci)
            cost = -np.concatenate(
                [hi[:, 0:6].reshape(4, CI, 6).sum(1, dtype=np.float32),
                 lo[:, 0:6].reshape(2, CI, 6).sum(1, dtype=np.float32)],
                axis=0)
            G = np.concatenate(
                [hi[:, 6:12].reshape(4, CI, 6).sum(1, dtype=np.float32),
                 lo[:, 6:12].reshape(2, CI, 6).sum(1, dtype=np.float32)],
                axis=0)

            totals = cost[t_idx, P].sum(-1, dtype=np.float32)
            perm = P[int(np.argmin(totals))]
            amask_b = -red[:, c0 + 24:c0 + 26].sum(dtype=np.float64)
            num += 0.5 * (amask_b - float(G[ar, perm].sum(dtype=np.float64)))

    den = float(m_np.sum())
    return np.float32(num / den)
